# revision 16
# baseline (speedup 1.0000x reference)
"""BKT forward kernel for Trainium2 (8 NeuronCores, data-parallel over batch).

Math: in odds space rho = L/(1-L) the BKT update is affine:
    rho' = a_t * rho + lam,   a_t = y ? (1-s)/(g(1-l)) : s/((1-g)(1-l)),
    lam = l/(1-l),
because the per-step Mobius map fixes L=1. The clip L <= 1-EPS becomes
rho <= R. Pin steps (where the clip binds) are detected with a log-depth
scan u_t = min(u_{t-1} + ln a_t, 0) (exact modulo the lam/R ~ 5e-4 shift,
threshold theta = -lam/R), then the trajectory is reconstructed with a
mult/add scan whose operands are masked to force state = R at pins.
Both recurrences are single tensor_tensor_scan instructions per
128-student tile, so the whole problem is a few big-FD ops per tile.
"""

import numpy as np

B_FULL = 65536
T = 512
N_CORES = 8
B_CORE = B_FULL // N_CORES          # 8192
N_TILES = B_CORE // 128             # 64
EPS = 1e-6

_cache = {}


def _consts():
    f32 = np.float32
    Lstar = f32(1.0) - f32(EPS)     # f32(1-1e-6)
    R = f32(np.float64(Lstar) / (1.0 - np.float64(Lstar)))
    return float(R)


def _build_bass():
    import concourse.bacc as bacc
    import concourse.mybir as mybir
    from concourse.tile import TileContext

    R = _consts()
    dt = mybir.dt
    op = mybir.AluOpType
    act = mybir.ActivationFunctionType

    nc = bacc.Bacc(None, target_bir_lowering=False)
    y_d = nc.dram_tensor("y", [B_CORE, T], dt.int32, kind="ExternalInput")
    par_d = nc.dram_tensor("par", [128, N_TILES * 8], dt.float32, kind="ExternalInput")
    lat_d = nc.dram_tensor("lat", [B_CORE, T], dt.float32, kind="ExternalOutput")
    cor_d = nc.dram_tensor("cor", [B_CORE, T], dt.float32, kind="ExternalOutput")

    with TileContext(nc) as tc:
        with (
            tc.tile_pool(name="const", bufs=1) as cpool,
            tc.tile_pool(name="work", bufs=3) as pool,
        ):
            zero_t = cpool.tile([128, T], dt.float32)
            nc.vector.memset(zero_t[:], 0.0)
            par_t = cpool.tile([128, N_TILES * 8], dt.float32)
            nc.gpsimd.dma_start(par_t[:], par_d[:, :])
            # per-engine copies so scalar-AP reads are same-engine deps
            # (avoids "too many sync wait commands" in walrus codegen)
            par_gp = cpool.tile([128, N_TILES * 8], dt.float32)
            nc.gpsimd.tensor_copy(par_gp[:], par_t[:])
            par_dv = cpool.tile([128, N_TILES * 8], dt.float32)
            nc.vector.tensor_copy(par_dv[:], par_t[:])
            par_ac = cpool.tile([128, N_TILES * 8], dt.float32)
            nc.scalar.copy(par_ac[:], par_t[:])

            for j in range(N_TILES):
                r0, r1 = j * 128, (j + 1) * 128
                b = j * 8
                y_t = pool.tile([128, T], dt.int32, tag="y")
                nc.sync.dma_start(y_t[:], y_d[r0:r1, :])

                yf = pool.tile([128, T], dt.float32, tag="yf")
                nc.gpsimd.tensor_copy(yf[:], y_t[:])
                a_t = pool.tile([128, T], dt.float32, tag="a")
                nc.scalar.activation(
                    a_t[:], yf[:], act.Identity,
                    bias=par_ac[:, b + 0 : b + 1], scale=par_ac[:, b + 1 : b + 2],
                )

                la_t = pool.tile([128, T], dt.float32, tag="la")
                nc.scalar.activation(la_t[:], a_t[:], act.Ln)

                u_t = pool.tile([128, T], dt.float32, tag="u")
                nc.vector.tensor_tensor_scan(
                    u_t[:], la_t[:], zero_t[:], par_dv[:, b + 7 : b + 8],
                    op.add, op.min,
                )

                notm = pool.tile([128, T], dt.float32, tag="notm")
                nc.gpsimd.tensor_scalar(
                    notm[:], u_t[:], par_gp[:, b + 6 : b + 7], None, op.is_lt
                )

                t1_t = pool.tile([128, T], dt.float32, tag="t1")
                nc.gpsimd.tensor_scalar_mul(
                    t1_t[:], notm[:], par_gp[:, b + 2 : b + 3]
                )
                m2_t = pool.tile([128, T], dt.float32, tag="m2")
                nc.gpsimd.tensor_scalar(m2_t[:], notm[:], -R, R, op.mult, op.add)

                d0_t = pool.tile([128, T], dt.float32, tag="d0")
                nc.vector.tensor_tensor(d0_t[:], a_t[:], notm[:], op.mult)
                d1_t = pool.tile([128, T], dt.float32, tag="d1")
                nc.vector.tensor_tensor(d1_t[:], t1_t[:], m2_t[:], op.add)

                p_t = pool.tile([128, T + 1], dt.float32, tag="p")
                nc.vector.tensor_copy(p_t[:, 0:1], par_dv[:, b + 5 : b + 6])
                nc.vector.tensor_tensor_scan(
                    p_t[:, 1 : T + 1], d0_t[:], d1_t[:],
                    par_dv[:, b + 5 : b + 6], op.mult, op.add,
                )

                q_t = pool.tile([128, T], dt.float32, tag="q")
                nc.vector.tensor_scalar(q_t[:], p_t[:, 0:T], R, 1.0, op.min, op.add)
                r_t = pool.tile([128, T], dt.float32, tag="r")
                nc.vector.reciprocal_approx_fast(r_t[:], q_t[:])

                lat_t = pool.tile([128, T], dt.float32, tag="lat")
                nc.gpsimd.tensor_scalar(
                    lat_t[:], r_t[:], -1.0, 1.0, op.mult, op.add
                )
                cor_t = pool.tile([128, T], dt.float32, tag="cor")
                nc.scalar.activation(
                    cor_t[:], r_t[:], act.Identity,
                    bias=par_ac[:, b + 4 : b + 5], scale=par_ac[:, b + 3 : b + 4],
                )

                nc.scalar.dma_start(lat_d[r0:r1, :], lat_t[:])
                nc.gpsimd.dma_start(cor_d[r0:r1, :], cor_t[:])
    nc.compile()
    return nc


def _host_params(X, learn_w, guess_w, slip_w, prior_w):
    f32 = np.float32
    f64 = np.float64

    def sig(w):
        return (1.0 / (1.0 + np.exp(-w.astype(f64)))).astype(f32)

    l = sig(learn_w[X[:, 0], 0])
    g = sig(guess_w[X[:, 1], 0])
    s = sig(slip_w[X[:, 2], 0])
    p = sig(prior_w[X[:, 3], 0])
    one = f32(1)
    R = f32(_consts())
    a1 = ((one - s) / (g * (one - l))).astype(f32)
    a0 = (s / ((one - g) * (one - l))).astype(f32)
    lam = (l / (one - l)).astype(f32)
    rho0 = (p / (one - p)).astype(f32)
    negc = (-(one - s - g)).astype(f32)
    ghat = (one - s).astype(f32)
    d = (a1 - a0).astype(f32)
    theta = (-(lam.astype(f64)) / f64(R)).astype(f32)
    u0 = np.log(rho0.astype(f64) / f64(R)).astype(f32)
    par = np.stack([a0, d, lam, negc, ghat, rho0, theta, u0], axis=1)
    # per-core layout (128, N_TILES*8): partition p, col j*8+k = student j*128+p
    par = par.reshape(N_CORES, N_TILES, 128, 8).transpose(0, 2, 1, 3)
    return np.ascontiguousarray(par.reshape(N_CORES, 128, N_TILES * 8), dtype=f32)


def kernel(X, y, learn_w, guess_w, slip_w, prior_w, _trace=False):
    from concourse import bass_utils

    X = np.asarray(X)
    y = np.ascontiguousarray(np.asarray(y, dtype=np.int32))
    par = _host_params(
        np.asarray(X),
        np.asarray(learn_w, np.float32),
        np.asarray(guess_w, np.float32),
        np.asarray(slip_w, np.float32),
        np.asarray(prior_w, np.float32),
    )

    if "nc" not in _cache:
        _cache["nc"] = _build_bass()
    nc = _cache["nc"]

    in_maps = [
        {"y": y[i * B_CORE : (i + 1) * B_CORE], "par": par[i]}
        for i in range(N_CORES)
    ]
    res = bass_utils.run_bass_kernel_spmd(
        nc, in_maps, core_ids=list(range(N_CORES)), trace=_trace
    )
    outs = res.results
    cor = np.concatenate([outs[i]["cor"] for i in range(N_CORES)], axis=0)
    lat = np.concatenate([outs[i]["lat"] for i in range(N_CORES)], axis=0)
    if _trace:
        _cache["last_exec_time_ns"] = res.exec_time_ns
    return cor, lat


# revision 19
# speedup vs baseline: 1.0266x; 1.0266x over previous
"""BKT forward kernel for Trainium2 (8 NeuronCores, data-parallel over batch).

Math: in odds space rho = L/(1-L) the BKT update is affine:
    rho' = a_t * rho + lam,   a_t = y ? (1-s)/(g(1-l)) : s/((1-g)(1-l)),
    lam = l/(1-l),
because the per-step Mobius map fixes L=1. The clip L <= 1-EPS becomes
rho <= R. Pin steps (where the clip binds) are detected with a log-depth
scan u_t = min(u_{t-1} + ln a_t, 0) (exact modulo the lam/R ~ 5e-4 shift,
threshold theta = -lam/R), then the trajectory is reconstructed with a
mult/add scan whose operands are masked to force state = R at pins.
Both recurrences are single tensor_tensor_scan instructions per
128-student tile, so the whole problem is a few big-FD ops per tile.
"""

import numpy as np

B_FULL = 65536
T = 512
N_CORES = 8
B_CORE = B_FULL // N_CORES          # 8192
N_TILES = B_CORE // 128             # 64
EPS = 1e-6

_cache = {}


def _consts():
    f32 = np.float32
    Lstar = f32(1.0) - f32(EPS)     # f32(1-1e-6)
    R = f32(np.float64(Lstar) / (1.0 - np.float64(Lstar)))
    return float(R)


def _build_bass():
    import concourse.bacc as bacc
    import concourse.mybir as mybir
    from concourse.tile import TileContext

    R = _consts()
    dt = mybir.dt
    op = mybir.AluOpType
    act = mybir.ActivationFunctionType

    nc = bacc.Bacc(None, target_bir_lowering=False)
    y_d = nc.dram_tensor("y", [B_CORE, T], dt.int8, kind="ExternalInput")
    par_d = nc.dram_tensor("par", [128, N_TILES * 8], dt.float32, kind="ExternalInput")
    lat_d = nc.dram_tensor("lat", [B_CORE, T], dt.float32, kind="ExternalOutput")
    cor_d = nc.dram_tensor("cor", [B_CORE, T], dt.float32, kind="ExternalOutput")

    with TileContext(nc) as tc:
        with (
            tc.tile_pool(name="const", bufs=1) as cpool,
            tc.tile_pool(name="work", bufs=4) as pool,
        ):
            zero_t = cpool.tile([128, T], dt.float32)
            nc.vector.memset(zero_t[:], 0.0)
            par_t = cpool.tile([128, N_TILES * 8], dt.float32)
            nc.sync.dma_start(par_t[:], par_d[:, :])
            # per-engine copies so scalar-AP reads are same-engine deps
            # (avoids "too many sync wait commands" in walrus codegen)
            par_gp = cpool.tile([128, N_TILES * 8], dt.float32)
            nc.gpsimd.tensor_copy(par_gp[:], par_t[:])
            par_dv = cpool.tile([128, N_TILES * 8], dt.float32)
            nc.vector.tensor_copy(par_dv[:], par_t[:])
            par_ac = cpool.tile([128, N_TILES * 8], dt.float32)
            nc.scalar.copy(par_ac[:], par_t[:])

            for j in range(N_TILES):
                r0, r1 = j * 128, (j + 1) * 128
                b = j * 8
                y_t = pool.tile([128, T], dt.int8, tag="y")
                nc.sync.dma_start(y_t[:], y_d[r0:r1, :])

                yf = pool.tile([128, T], dt.float32, tag="yf")
                nc.gpsimd.tensor_copy(yf[:], y_t[:])
                a_t = pool.tile([128, T], dt.float32, tag="a")
                nc.scalar.activation(
                    a_t[:], yf[:], act.Identity,
                    bias=par_ac[:, b + 0 : b + 1], scale=par_ac[:, b + 1 : b + 2],
                )

                la_t = pool.tile([128, T], dt.float32, tag="la")
                nc.scalar.activation(la_t[:], a_t[:], act.Ln)

                u_t = pool.tile([128, T], dt.float32, tag="u")
                nc.vector.tensor_tensor_scan(
                    u_t[:], la_t[:], zero_t[:], par_dv[:, b + 7 : b + 8],
                    op.add, op.min,
                )

                notm = pool.tile([128, T], dt.float32, tag="notm")
                nc.gpsimd.tensor_scalar(
                    notm[:], u_t[:], par_gp[:, b + 6 : b + 7], None, op.is_lt
                )

                t1_t = pool.tile([128, T], dt.float32, tag="t1")
                nc.gpsimd.tensor_scalar_mul(
                    t1_t[:], notm[:], par_gp[:, b + 2 : b + 3]
                )
                m2_t = pool.tile([128, T], dt.float32, tag="m2")
                nc.gpsimd.tensor_scalar(m2_t[:], notm[:], -R, R, op.mult, op.add)

                d0_t = pool.tile([128, T], dt.float32, tag="d0")
                nc.vector.tensor_tensor(d0_t[:], a_t[:], notm[:], op.mult)
                d1_t = pool.tile([128, T], dt.float32, tag="d1")
                nc.vector.tensor_tensor(d1_t[:], t1_t[:], m2_t[:], op.add)

                p_t = pool.tile([128, T + 1], dt.float32, tag="p")
                nc.vector.tensor_copy(p_t[:, 0:1], par_dv[:, b + 5 : b + 6])
                nc.vector.tensor_tensor_scan(
                    p_t[:, 1 : T + 1], d0_t[:], d1_t[:],
                    par_dv[:, b + 5 : b + 6], op.mult, op.add,
                )

                q_t = pool.tile([128, T], dt.float32, tag="q")
                nc.gpsimd.tensor_scalar(q_t[:], p_t[:, 0:T], R, 1.0, op.min, op.add)
                r_t = pool.tile([128, T], dt.float32, tag="r")
                nc.vector.reciprocal_approx_fast(r_t[:], q_t[:])

                lat_t = pool.tile([128, T], dt.float32, tag="lat")
                nc.scalar.activation(
                    lat_t[:], r_t[:], act.Identity, bias=1.0, scale=-1.0
                )
                cor_t = pool.tile([128, T], dt.float32, tag="cor")
                nc.scalar.activation(
                    cor_t[:], r_t[:], act.Identity,
                    bias=par_ac[:, b + 4 : b + 5], scale=par_ac[:, b + 3 : b + 4],
                )

                if j % 2 == 0:
                    nc.scalar.dma_start(lat_d[r0:r1, :], lat_t[:])
                    nc.gpsimd.dma_start(cor_d[r0:r1, :], cor_t[:])
                else:
                    nc.sync.dma_start(lat_d[r0:r1, :], lat_t[:])
                    nc.scalar.dma_start(cor_d[r0:r1, :], cor_t[:])
    nc.compile()
    return nc


def _host_params(X, learn_w, guess_w, slip_w, prior_w):
    f32 = np.float32
    f64 = np.float64

    def sig(w):
        return (1.0 / (1.0 + np.exp(-w.astype(f64)))).astype(f32)

    l = sig(learn_w[X[:, 0], 0])
    g = sig(guess_w[X[:, 1], 0])
    s = sig(slip_w[X[:, 2], 0])
    p = sig(prior_w[X[:, 3], 0])
    one = f32(1)
    R = f32(_consts())
    a1 = ((one - s) / (g * (one - l))).astype(f32)
    a0 = (s / ((one - g) * (one - l))).astype(f32)
    lam = (l / (one - l)).astype(f32)
    rho0 = (p / (one - p)).astype(f32)
    negc = (-(one - s - g)).astype(f32)
    ghat = (one - s).astype(f32)
    d = (a1 - a0).astype(f32)
    theta = (-(lam.astype(f64)) / f64(R)).astype(f32)
    u0 = np.log(rho0.astype(f64) / f64(R)).astype(f32)
    par = np.stack([a0, d, lam, negc, ghat, rho0, theta, u0], axis=1)
    # per-core layout (128, N_TILES*8): partition p, col j*8+k = student j*128+p
    par = par.reshape(N_CORES, N_TILES, 128, 8).transpose(0, 2, 1, 3)
    return np.ascontiguousarray(par.reshape(N_CORES, 128, N_TILES * 8), dtype=f32)


def kernel(X, y, learn_w, guess_w, slip_w, prior_w, _trace=False):
    from concourse import bass_utils

    X = np.asarray(X)
    y = np.ascontiguousarray(np.asarray(y, dtype=np.int8))
    par = _host_params(
        np.asarray(X),
        np.asarray(learn_w, np.float32),
        np.asarray(guess_w, np.float32),
        np.asarray(slip_w, np.float32),
        np.asarray(prior_w, np.float32),
    )

    if "nc" not in _cache:
        _cache["nc"] = _build_bass()
    nc = _cache["nc"]

    in_maps = [
        {"y": y[i * B_CORE : (i + 1) * B_CORE], "par": par[i]}
        for i in range(N_CORES)
    ]
    res = bass_utils.run_bass_kernel_spmd(
        nc, in_maps, core_ids=list(range(N_CORES)), trace=_trace
    )
    outs = res.results
    cor = np.concatenate([outs[i]["cor"] for i in range(N_CORES)], axis=0)
    lat = np.concatenate([outs[i]["lat"] for i in range(N_CORES)], axis=0)
    if _trace:
        _cache["last_exec_time_ns"] = res.exec_time_ns
    return cor, lat


# revision 20
# speedup vs baseline: 1.1052x; 1.0766x over previous
"""BKT forward kernel for Trainium2 (8 NeuronCores, data-parallel over batch).

Math: in odds space rho = L/(1-L) the BKT update is affine:
    rho' = a_t * rho + lam,   a_t = y ? (1-s)/(g(1-l)) : s/((1-g)(1-l)),
    lam = l/(1-l),
because the per-step Mobius map fixes L=1. The clip L <= 1-EPS becomes
rho <= R. Pin steps (where the clip binds) are detected with a log-depth
scan u_t = min(u_{t-1} + ln a_t, 0) (exact modulo the lam/R ~ 5e-4 shift,
threshold theta = -lam/R), then the trajectory is reconstructed with a
mult/add scan whose operands are masked to force state = R at pins.
Both recurrences are single tensor_tensor_scan instructions per
128-student tile, so the whole problem is a few big-FD ops per tile.
"""

import numpy as np

B_FULL = 65536
T = 512
N_CORES = 8
B_CORE = B_FULL // N_CORES          # 8192
N_TILES = B_CORE // 128             # 64
EPS = 1e-6

_cache = {}


def _consts():
    f32 = np.float32
    Lstar = f32(1.0) - f32(EPS)     # f32(1-1e-6)
    R = f32(np.float64(Lstar) / (1.0 - np.float64(Lstar)))
    return float(R)


def _build_bass():
    import concourse.bacc as bacc
    import concourse.mybir as mybir
    from concourse.tile import TileContext

    R = _consts()
    dt = mybir.dt
    op = mybir.AluOpType
    act = mybir.ActivationFunctionType

    nc = bacc.Bacc(None, target_bir_lowering=False)
    y_d = nc.dram_tensor("y", [B_CORE, T], dt.int8, kind="ExternalInput")
    par_d = nc.dram_tensor("par", [128, N_TILES * 8], dt.float32, kind="ExternalInput")
    lat_d = nc.dram_tensor("lat", [B_CORE, T], dt.float32, kind="ExternalOutput")
    cor_d = nc.dram_tensor("cor", [B_CORE, T], dt.float32, kind="ExternalOutput")

    with TileContext(nc) as tc:
        with (
            tc.tile_pool(name="const", bufs=1) as cpool,
            tc.tile_pool(name="work", bufs=4) as pool,
        ):
            zero_t = cpool.tile([128, T], dt.float32)
            nc.vector.memset(zero_t[:], 0.0)
            par_t = cpool.tile([128, N_TILES * 8], dt.float32)
            nc.sync.dma_start(par_t[:], par_d[:, :])
            # per-engine copies so scalar-AP reads are same-engine deps
            # (avoids "too many sync wait commands" in walrus codegen)
            par_gp = cpool.tile([128, N_TILES * 8], dt.float32)
            nc.gpsimd.tensor_copy(par_gp[:], par_t[:])
            par_dv = cpool.tile([128, N_TILES * 8], dt.float32)
            nc.vector.tensor_copy(par_dv[:], par_t[:])
            par_ac = cpool.tile([128, N_TILES * 8], dt.float32)
            nc.scalar.copy(par_ac[:], par_t[:])

            for j in range(N_TILES):
                r0, r1 = j * 128, (j + 1) * 128
                b = j * 8
                y_t = pool.tile([128, T], dt.int8, tag="y")
                nc.sync.dma_start(y_t[:], y_d[r0:r1, :])

                a_t = pool.tile([128, T], dt.float32, tag="a")
                nc.scalar.activation(
                    a_t[:], y_t[:], act.Identity,
                    bias=par_ac[:, b + 0 : b + 1], scale=par_ac[:, b + 1 : b + 2],
                )

                la_t = pool.tile([128, T], dt.float32, tag="la")
                nc.scalar.activation(la_t[:], a_t[:], act.Ln)

                u_t = pool.tile([128, T], dt.float32, tag="u")
                nc.vector.tensor_tensor_scan(
                    u_t[:], la_t[:], zero_t[:], par_dv[:, b + 7 : b + 8],
                    op.add, op.min,
                )

                notm = pool.tile([128, T], dt.float32, tag="notm")
                nc.gpsimd.tensor_scalar(
                    notm[:], u_t[:], par_gp[:, b + 6 : b + 7], None, op.is_lt
                )

                t1_t = pool.tile([128, T], dt.float32, tag="t1")
                nc.gpsimd.tensor_scalar_mul(
                    t1_t[:], notm[:], par_gp[:, b + 2 : b + 3]
                )
                m2_t = pool.tile([128, T], dt.float32, tag="m2")
                nc.gpsimd.tensor_scalar(m2_t[:], notm[:], -R, R, op.mult, op.add)

                d0_t = pool.tile([128, T], dt.float32, tag="d0")
                nc.vector.tensor_tensor(d0_t[:], a_t[:], notm[:], op.mult)
                d1_t = pool.tile([128, T], dt.float32, tag="d1")
                nc.vector.tensor_tensor(d1_t[:], t1_t[:], m2_t[:], op.add)

                p_t = pool.tile([128, T + 1], dt.float32, tag="p")
                nc.vector.tensor_copy(p_t[:, 0:1], par_dv[:, b + 5 : b + 6])
                nc.vector.tensor_tensor_scan(
                    p_t[:, 1 : T + 1], d0_t[:], d1_t[:],
                    par_dv[:, b + 5 : b + 6], op.mult, op.add,
                )

                q_t = pool.tile([128, T], dt.float32, tag="q")
                nc.gpsimd.tensor_scalar(q_t[:], p_t[:, 0:T], R, 1.0, op.min, op.add)
                r_t = pool.tile([128, T], dt.float32, tag="r")
                nc.vector.reciprocal_approx_fast(r_t[:], q_t[:])

                lat_t = pool.tile([128, T], dt.float32, tag="lat")
                nc.scalar.activation(
                    lat_t[:], r_t[:], act.Identity, bias=1.0, scale=-1.0
                )
                cor_t = pool.tile([128, T], dt.float32, tag="cor")
                nc.scalar.activation(
                    cor_t[:], r_t[:], act.Identity,
                    bias=par_ac[:, b + 4 : b + 5], scale=par_ac[:, b + 3 : b + 4],
                )

                lat_q = [nc.scalar, nc.sync, nc.gpsimd][j % 3]
                cor_q = [nc.gpsimd, nc.scalar, nc.sync][j % 3]
                lat_q.dma_start(lat_d[r0:r1, :], lat_t[:])
                cor_q.dma_start(cor_d[r0:r1, :], cor_t[:])
    nc.compile()
    return nc


def _host_params(X, learn_w, guess_w, slip_w, prior_w):
    f32 = np.float32
    f64 = np.float64

    def sig(w):
        return (1.0 / (1.0 + np.exp(-w.astype(f64)))).astype(f32)

    l = sig(learn_w[X[:, 0], 0])
    g = sig(guess_w[X[:, 1], 0])
    s = sig(slip_w[X[:, 2], 0])
    p = sig(prior_w[X[:, 3], 0])
    one = f32(1)
    R = f32(_consts())
    a1 = ((one - s) / (g * (one - l))).astype(f32)
    a0 = (s / ((one - g) * (one - l))).astype(f32)
    lam = (l / (one - l)).astype(f32)
    rho0 = (p / (one - p)).astype(f32)
    negc = (-(one - s - g)).astype(f32)
    ghat = (one - s).astype(f32)
    d = (a1 - a0).astype(f32)
    theta = (-(lam.astype(f64)) / f64(R)).astype(f32)
    u0 = np.log(rho0.astype(f64) / f64(R)).astype(f32)
    par = np.stack([a0, d, lam, negc, ghat, rho0, theta, u0], axis=1)
    # per-core layout (128, N_TILES*8): partition p, col j*8+k = student j*128+p
    par = par.reshape(N_CORES, N_TILES, 128, 8).transpose(0, 2, 1, 3)
    return np.ascontiguousarray(par.reshape(N_CORES, 128, N_TILES * 8), dtype=f32)


def kernel(X, y, learn_w, guess_w, slip_w, prior_w, _trace=False):
    from concourse import bass_utils

    X = np.asarray(X)
    y = np.ascontiguousarray(np.asarray(y, dtype=np.int8))
    par = _host_params(
        np.asarray(X),
        np.asarray(learn_w, np.float32),
        np.asarray(guess_w, np.float32),
        np.asarray(slip_w, np.float32),
        np.asarray(prior_w, np.float32),
    )

    if "nc" not in _cache:
        _cache["nc"] = _build_bass()
    nc = _cache["nc"]

    in_maps = [
        {"y": y[i * B_CORE : (i + 1) * B_CORE], "par": par[i]}
        for i in range(N_CORES)
    ]
    res = bass_utils.run_bass_kernel_spmd(
        nc, in_maps, core_ids=list(range(N_CORES)), trace=_trace
    )
    outs = res.results
    cor = np.concatenate([outs[i]["cor"] for i in range(N_CORES)], axis=0)
    lat = np.concatenate([outs[i]["lat"] for i in range(N_CORES)], axis=0)
    if _trace:
        _cache["last_exec_time_ns"] = res.exec_time_ns
    return cor, lat


# revision 22
# speedup vs baseline: 1.1087x; 1.0032x over previous
"""BKT forward kernel for Trainium2 (8 NeuronCores, data-parallel over batch).

Math: in odds space rho = L/(1-L) the BKT update is affine:
    rho' = a_t * rho + lam,   a_t = y ? (1-s)/(g(1-l)) : s/((1-g)(1-l)),
    lam = l/(1-l),
because the per-step Mobius map fixes L=1. The clip L <= 1-EPS becomes
rho <= R. Pin steps (where the clip binds) are detected with a log-depth
scan u_t = min(u_{t-1} + ln a_t, 0) (exact modulo the lam/R ~ 5e-4 shift,
threshold theta = -lam/R), then the trajectory is reconstructed with a
mult/add scan whose operands are masked to force state = R at pins.
Both recurrences are single tensor_tensor_scan instructions per
128-student tile, so the whole problem is a few big-FD ops per tile.
"""

import numpy as np

B_FULL = 65536
T = 512
N_CORES = 8
B_CORE = B_FULL // N_CORES          # 8192
N_TILES = B_CORE // 128             # 64
EPS = 1e-6

_cache = {}


def _consts():
    f32 = np.float32
    Lstar = f32(1.0) - f32(EPS)     # f32(1-1e-6)
    R = f32(np.float64(Lstar) / (1.0 - np.float64(Lstar)))
    return float(R)


def _build_bass():
    import concourse.bacc as bacc
    import concourse.mybir as mybir
    from concourse.tile import TileContext

    R = _consts()
    dt = mybir.dt
    op = mybir.AluOpType
    act = mybir.ActivationFunctionType

    nc = bacc.Bacc(None, target_bir_lowering=False)
    y_d = nc.dram_tensor("y", [B_CORE, T], dt.int8, kind="ExternalInput")
    par_d = nc.dram_tensor("par", [128, N_TILES * 8], dt.float32, kind="ExternalInput")
    lat_d = nc.dram_tensor("lat", [B_CORE, T], dt.float32, kind="ExternalOutput")
    cor_d = nc.dram_tensor("cor", [B_CORE, T], dt.float32, kind="ExternalOutput")

    with TileContext(nc) as tc:
        with (
            tc.tile_pool(name="const", bufs=1) as cpool,
            tc.tile_pool(name="work", bufs=4) as pool,
        ):
            zero_t = cpool.tile([128, T], dt.float32)
            nc.vector.memset(zero_t[:], 0.0)
            par_t = cpool.tile([128, N_TILES * 8], dt.float32)
            nc.sync.dma_start(par_t[:], par_d[:, :])
            # per-engine copies so scalar-AP reads are same-engine deps
            # (avoids "too many sync wait commands" in walrus codegen)
            par_gp = cpool.tile([128, N_TILES * 8], dt.float32)
            nc.gpsimd.tensor_copy(par_gp[:], par_t[:])
            par_dv = cpool.tile([128, N_TILES * 8], dt.float32)
            nc.vector.tensor_copy(par_dv[:], par_t[:])
            par_ac = cpool.tile([128, N_TILES * 8], dt.float32)
            nc.scalar.copy(par_ac[:], par_t[:])

            for j in range(N_TILES):
                r0, r1 = j * 128, (j + 1) * 128
                b = j * 8
                y_t = pool.tile([128, T], dt.int8, tag="y")
                nc.sync.dma_start(y_t[:], y_d[r0:r1, :])

                a_t = pool.tile([128, T], dt.float32, tag="a")
                nc.scalar.activation(
                    a_t[:], y_t[:], act.Identity,
                    bias=par_ac[:, b + 0 : b + 1], scale=par_ac[:, b + 1 : b + 2],
                )

                la_t = pool.tile([128, T], dt.float32, tag="la")
                nc.scalar.activation(la_t[:], a_t[:], act.Ln)

                u_t = pool.tile([128, T], dt.float32, tag="u")
                nc.vector.tensor_tensor_scan(
                    u_t[:], la_t[:], zero_t[:], par_dv[:, b + 7 : b + 8],
                    op.add, op.min,
                )

                notm = pool.tile([128, T], dt.float32, tag="notm")
                nc.gpsimd.tensor_scalar(
                    notm[:], u_t[:], par_gp[:, b + 6 : b + 7], None, op.is_lt
                )

                m2_t = pool.tile([128, T], dt.float32, tag="m2")
                nc.gpsimd.tensor_scalar(m2_t[:], notm[:], -R, R, op.mult, op.add)

                d0_t = pool.tile([128, T], dt.float32, tag="d0")
                nc.vector.tensor_tensor(d0_t[:], a_t[:], notm[:], op.mult)
                d1_t = pool.tile([128, T], dt.float32, tag="d1")
                nc.vector.scalar_tensor_tensor(
                    d1_t[:], notm[:], par_dv[:, b + 2 : b + 3], m2_t[:],
                    op.mult, op.add,
                )

                p_t = pool.tile([128, T + 1], dt.float32, tag="p")
                nc.vector.tensor_copy(p_t[:, 0:1], par_dv[:, b + 5 : b + 6])
                nc.vector.tensor_tensor_scan(
                    p_t[:, 1 : T + 1], d0_t[:], d1_t[:],
                    par_dv[:, b + 5 : b + 6], op.mult, op.add,
                )

                q_t = pool.tile([128, T], dt.float32, tag="q")
                nc.gpsimd.tensor_scalar(q_t[:], p_t[:, 0:T], R, 1.0, op.min, op.add)
                r_t = pool.tile([128, T], dt.float32, tag="r")
                nc.vector.reciprocal_approx_fast(r_t[:], q_t[:])

                lat_t = pool.tile([128, T], dt.float32, tag="lat")
                nc.scalar.activation(
                    lat_t[:], r_t[:], act.Identity, bias=1.0, scale=-1.0
                )
                cor_t = pool.tile([128, T], dt.float32, tag="cor")
                nc.scalar.activation(
                    cor_t[:], r_t[:], act.Identity,
                    bias=par_ac[:, b + 4 : b + 5], scale=par_ac[:, b + 3 : b + 4],
                )

                lat_q = [nc.scalar, nc.sync, nc.gpsimd][j % 3]
                cor_q = [nc.gpsimd, nc.scalar, nc.sync][j % 3]
                lat_q.dma_start(lat_d[r0:r1, :], lat_t[:])
                cor_q.dma_start(cor_d[r0:r1, :], cor_t[:])
    nc.compile()
    return nc


def _host_params(X, learn_w, guess_w, slip_w, prior_w):
    f32 = np.float32
    f64 = np.float64

    def sig(w):
        return (1.0 / (1.0 + np.exp(-w.astype(f64)))).astype(f32)

    l = sig(learn_w[X[:, 0], 0])
    g = sig(guess_w[X[:, 1], 0])
    s = sig(slip_w[X[:, 2], 0])
    p = sig(prior_w[X[:, 3], 0])
    one = f32(1)
    R = f32(_consts())
    a1 = ((one - s) / (g * (one - l))).astype(f32)
    a0 = (s / ((one - g) * (one - l))).astype(f32)
    lam = (l / (one - l)).astype(f32)
    rho0 = (p / (one - p)).astype(f32)
    negc = (-(one - s - g)).astype(f32)
    ghat = (one - s).astype(f32)
    d = (a1 - a0).astype(f32)
    theta = (-(lam.astype(f64)) / f64(R)).astype(f32)
    u0 = np.log(rho0.astype(f64) / f64(R)).astype(f32)
    par = np.stack([a0, d, lam, negc, ghat, rho0, theta, u0], axis=1)
    # per-core layout (128, N_TILES*8): partition p, col j*8+k = student j*128+p
    par = par.reshape(N_CORES, N_TILES, 128, 8).transpose(0, 2, 1, 3)
    return np.ascontiguousarray(par.reshape(N_CORES, 128, N_TILES * 8), dtype=f32)


def kernel(X, y, learn_w, guess_w, slip_w, prior_w, _trace=False):
    from concourse import bass_utils

    X = np.asarray(X)
    y = np.ascontiguousarray(np.asarray(y, dtype=np.int8))
    par = _host_params(
        np.asarray(X),
        np.asarray(learn_w, np.float32),
        np.asarray(guess_w, np.float32),
        np.asarray(slip_w, np.float32),
        np.asarray(prior_w, np.float32),
    )

    if "nc" not in _cache:
        _cache["nc"] = _build_bass()
    nc = _cache["nc"]

    in_maps = [
        {"y": y[i * B_CORE : (i + 1) * B_CORE], "par": par[i]}
        for i in range(N_CORES)
    ]
    res = bass_utils.run_bass_kernel_spmd(
        nc, in_maps, core_ids=list(range(N_CORES)), trace=_trace
    )
    outs = res.results
    cor = np.concatenate([outs[i]["cor"] for i in range(N_CORES)], axis=0)
    lat = np.concatenate([outs[i]["lat"] for i in range(N_CORES)], axis=0)
    if _trace:
        _cache["last_exec_time_ns"] = res.exec_time_ns
    return cor, lat
